# revision 9
# baseline (speedup 1.0000x reference)
"""Trainium2 Bass kernel for nn_NeptuneMoEModel (moe_routing).

Model: 6 small MLPs (router + 2 energy experts + 3 direction experts) over
N=262144 points -> segment-mean-pool into B=1024 events -> tiny per-event
head/mixing math.

Strategy (8 NeuronCores, SPMD, data-parallel over events):
  - Events sorted by point count and round-robin assigned to cores so slot s
    on every core holds a similarly-sized event; slot lengths are uniform
    across cores (required: one program for all 8 cores).
  - Slots first-fit packed into 1024-column "windows" (= 2 PSUM banks).
  - Feature-major layout on device: x as [9, S]; layer1 = fused [9, 1536]
    matmul, layer2 = 6x [256,256], all in float32r (1 cyc/row at N=512).
  - All 6 heads fused into one block-diagonal [1536 -> 19] matmul that
    accumulates in PSUM (12 accumulating matmuls per window); pooling then
    reduces only [19, L] per event on the vector engine.
  - gelu (tanh approx, matches jax.nn.gelu) via big [128, 1024] scalar-engine
    activations reading PSUM directly, per-partition bias APs.
  - Host: pad-correction (exact, general for nonzero biases), divide by
    counts, head biases, softmax/gating mixing - all O(B*19) numpy.
"""

import sys

sys.path.insert(0, "/opt/trn_rl_repo")

import numpy as np

import concourse.bass as bass
import concourse.mybir as mybir
import concourse.tile as tile
from concourse import bacc

N_CORES = 8
B = 1024
N_PTS = 262144
DIN = 9
H = 256
NNETS = 6
ZDIMS = [6, 2, 2, 3, 3, 3]
ZOFF = [0, 6, 8, 10, 13, 16]
ZD = 19
WIN = 1024
PIECE = 512
SLOTS = B // N_CORES  # 128
F32 = mybir.dt.float32
BF16 = mybir.dt.bfloat16
try:
    import ml_dtypes

    NPBF16 = ml_dtypes.bfloat16
except ImportError:  # pragma: no cover
    NPBF16 = None
GELU = mybir.ActivationFunctionType.Gelu_apprx_tanh


def _gelu(x):
    """jax.nn.gelu(approximate=True) in numpy/fp32."""
    x = np.asarray(x, np.float32)
    c = np.float32(np.sqrt(2.0 / np.pi))
    return (0.5 * x * (1.0 + np.tanh(c * (x + 0.044715 * x * x * x)))).astype(
        np.float32
    )


# ----------------------------------------------------------------------------
# Layout: event -> (core, slot); slots -> windows
# ----------------------------------------------------------------------------


def build_layout(counts):
    counts = np.asarray(counts)
    order = np.argsort(-counts, kind="stable")
    ev = order.reshape(SLOTS, N_CORES)  # ev[s, c] = event id
    slot_len = counts[ev].max(1)
    slot_len = np.maximum(((slot_len + 3) // 4) * 4, 4).astype(np.int64)
    assert slot_len.max() <= WIN
    # first-fit (slot_len is non-increasing -> this is first-fit-decreasing)
    win_used = []
    slot_win = np.zeros(SLOTS, np.int64)
    slot_off = np.zeros(SLOTS, np.int64)
    for s in range(SLOTS):
        L = int(slot_len[s])
        for w in range(len(win_used)):
            if win_used[w] + L <= WIN:
                slot_win[s] = w
                slot_off[s] = win_used[w]
                win_used[w] += L
                break
        else:
            slot_win[s] = len(win_used)
            slot_off[s] = 0
            win_used.append(L)
    nw = len(win_used)
    slots_per_win = [[] for _ in range(nw)]
    for s in range(SLOTS):
        slots_per_win[slot_win[s]].append(
            (s, int(slot_off[s]), int(slot_len[s]))
        )
    return dict(
        ev=ev,
        slot_len=slot_len,
        slot_win=slot_win,
        slot_off=slot_off,
        nw=nw,
        slots_per_win=slots_per_win,
    )


# ----------------------------------------------------------------------------
# Device program
# ----------------------------------------------------------------------------


def build_program(nw, slots_per_win, slots=SLOTS, act=GELU):
    nc = bacc.Bacc(None, target_bir_lowering=False)
    S = nw * WIN
    xin = nc.dram_tensor("xin", [DIN, S], BF16, kind="ExternalInput")
    w1 = nc.dram_tensor("w1", [DIN, 12 * 128], BF16, kind="ExternalInput")
    w2a = nc.dram_tensor("w2a", [128, NNETS * 256], BF16, kind="ExternalInput")
    w2b = nc.dram_tensor("w2b", [128, NNETS * 256], BF16, kind="ExternalInput")
    whbd = nc.dram_tensor("whbd", [128, 12 * ZD], BF16, kind="ExternalInput")
    b1 = nc.dram_tensor("b1", [128, 12], F32, kind="ExternalInput")
    b2 = nc.dram_tensor("b2", [128, 12], F32, kind="ExternalInput")
    outt = nc.dram_tensor("zsum", [ZD, slots], F32, kind="ExternalOutput")

    npiece = WIN // PIECE
    with tile.TileContext(nc) as tc:
        with (
            tc.tile_pool(name="wts", bufs=1) as wts,
            tc.tile_pool(name="xp", bufs=3) as xp,
            tc.tile_pool(name="h1p", bufs=14) as h1p,
            tc.tile_pool(name="h2p", bufs=4) as h2p,
            tc.tile_pool(name="op", bufs=1) as op,
            tc.tile_pool(name="psm", bufs=2, space="PSUM") as psm,
            tc.tile_pool(name="psz", bufs=2, space="PSUM") as psz,
        ):
            w1t = wts.tile([DIN, 12 * 128], BF16)
            nc.sync.dma_start(w1t, w1[:, :])
            w2t = [wts.tile([128, NNETS * 256], BF16, name=f"w2_{k}") for k in range(2)]
            nc.sync.dma_start(w2t[0], w2a[:, :])
            nc.sync.dma_start(w2t[1], w2b[:, :])
            whbdt = wts.tile([128, 12 * ZD], BF16)
            nc.sync.dma_start(whbdt, whbd[:, :])
            b1t = wts.tile([128, 12], F32)
            nc.sync.dma_start(b1t, b1[:, :])
            b2t = wts.tile([128, 12], F32)
            nc.sync.dma_start(b2t, b2[:, :])
            zsb = op.tile([ZD, slots], F32)

            for w in range(nw):
                xw = xp.tile([DIN, WIN], BF16, tag="xw")
                nc.sync.dma_start(xw, xin[:, w * WIN : (w + 1) * WIN])
                # layer 1: h1[j] = gelu(x @ W1[:, j-tile] + b1), j = net*2 + half
                h1 = []
                for j in range(12):
                    ps = psm.tile([128, WIN], F32, tag="ps")
                    for p in range(npiece):
                        sl = slice(p * PIECE, (p + 1) * PIECE)
                        nc.tensor.matmul(
                            ps[:, sl],
                            w1t[:, j * 128 : (j + 1) * 128],
                            xw[:, sl],
                            start=True,
                            stop=True,
                        )
                    t = h1p.tile([128, WIN], BF16, tag="h1")
                    nc.scalar.activation(t, ps, act, bias=b1t[:, j : j + 1])
                    h1.append(t)
                # layer 2 + block-diagonal head accumulated into z psum
                z = psz.tile([ZD, WIN], F32, tag="z")
                for n in range(NNETS):
                    h2 = []
                    for mo in range(2):
                        j = 2 * n + mo
                        ps = psm.tile([128, WIN], F32, tag="ps")
                        c0 = n * 256 + mo * 128
                        for k in range(2):
                            for p in range(npiece):
                                sl = slice(p * PIECE, (p + 1) * PIECE)
                                nc.tensor.matmul(
                                    ps[:, sl],
                                    w2t[k][:, c0 : c0 + 128],
                                    h1[2 * n + k][:, sl],
                                    start=(k == 0),
                                    stop=(k == 1),
                                    skip_group_check=True,
                                )
                        t = h2p.tile([128, WIN], BF16, tag="h2")
                        nc.scalar.activation(t, ps, act, bias=b2t[:, j : j + 1])
                        h2.append(t)
                    for mo in range(2):
                        j = 2 * n + mo
                        for p in range(npiece):
                            sl = slice(p * PIECE, (p + 1) * PIECE)
                            nc.tensor.matmul(
                                z[:, sl],
                                whbdt[:, j * ZD : (j + 1) * ZD],
                                h2[mo][:, sl],
                                start=(j == 0),
                                stop=(j == 11),
                                skip_group_check=True,
                            )
                # per-event pooling: sum z columns of each slot
                for s, off, L in slots_per_win[w]:
                    nc.vector.tensor_reduce(
                        zsb[:, s : s + 1],
                        z[:, off : off + L],
                        axis=mybir.AxisListType.X,
                        op=mybir.AluOpType.add,
                    )
            nc.sync.dma_start(outt[:, :], zsb)
    nc.compile()
    return nc


# ----------------------------------------------------------------------------
# Host-side weight packing
# ----------------------------------------------------------------------------


def pack_weights(ins):
    W1s = [ins["router_W1"]] + [ins["e_W1"][i] for i in range(2)] + [
        ins["d_W1"][i] for i in range(3)
    ]
    W2s = [ins["router_W2"]] + [ins["e_W2"][i] for i in range(2)] + [
        ins["d_W2"][i] for i in range(3)
    ]
    Whs = [ins["router_Wh"]] + [ins["e_Wh"][i] for i in range(2)] + [
        ins["d_Wh"][i] for i in range(3)
    ]
    b1s = [ins["router_b1"]] + [ins["e_b1"][i] for i in range(2)] + [
        ins["d_b1"][i] for i in range(3)
    ]
    b2s = [ins["router_b2"]] + [ins["e_b2"][i] for i in range(2)] + [
        ins["d_b2"][i] for i in range(3)
    ]
    bhs = [ins["router_bh"]] + [ins["e_bh"][i] for i in range(2)] + [
        ins["d_bh"][i] for i in range(3)
    ]
    f = lambda a: np.ascontiguousarray(np.asarray(a, np.float32))
    W1cat = np.concatenate([f(w) for w in W1s], axis=1)  # [9, 1536]
    w2a = np.concatenate([f(w)[0:128, :] for w in W2s], axis=1)  # [128, 1536]
    w2b = np.concatenate([f(w)[128:256, :] for w in W2s], axis=1)
    b1cat = np.concatenate([f(b) for b in b1s])  # [1536]
    b2cat = np.concatenate([f(b) for b in b2s])
    bhcat = np.concatenate([f(b) for b in bhs])  # [19]
    whbd = np.zeros((128, 12 * ZD), np.float32)
    for j in range(12):
        nt, hj = j // 2, j % 2
        whbd[:, j * ZD + ZOFF[nt] : j * ZD + ZOFF[nt] + ZDIMS[nt]] = f(Whs[nt])[
            hj * 128 : (hj + 1) * 128, :
        ]
    b1t = b1cat.reshape(12, 128).T.copy()  # [128, 12]
    b2t = b2cat.reshape(12, 128).T.copy()
    # pad-column contribution (exact; zero when biases are zero)
    h1c = _gelu(b1cat)
    zc = np.zeros(ZD, np.float32)
    for n in range(NNETS):
        a2c = h1c[n * 256 : (n + 1) * 256] @ f(W2s[n]) + f(b2s[n])
        h2c = _gelu(a2c)
        zc[ZOFF[n] : ZOFF[n] + ZDIMS[n]] = h2c @ f(Whs[n])
    bf = lambda a: a.astype(NPBF16)
    return dict(
        w1=bf(W1cat), w2a=bf(w2a), w2b=bf(w2b), whbd=bf(whbd), b1=b1t, b2=b2t,
        bhcat=bhcat,
        zc=zc, W2s=[f(w) for w in W2s], Whs=[f(w) for w in Whs],
        b2s=[f(b) for b in b2s],
    )


def build_xall(x, batch_ids, lay):
    """Scatter points into per-core feature-major padded streams [8, 9, S]."""
    counts = np.bincount(batch_ids, minlength=B)
    seg_start = np.zeros(B, np.int64)
    np.cumsum(counts[:-1], out=seg_start[1:])
    rank = np.empty(B, np.int64)
    rank[lay["ev"].reshape(-1)] = np.arange(B)
    r = rank[batch_ids]
    s = r // N_CORES
    c = r % N_CORES
    pos = np.arange(N_PTS) - seg_start[batch_ids]
    col = lay["slot_win"][s] * WIN + lay["slot_off"][s] + pos
    S = lay["nw"] * WIN
    xall = np.zeros((N_CORES, DIN, S), NPBF16)
    xall[c, :, col] = x.astype(NPBF16)
    return xall


# ----------------------------------------------------------------------------
# Host-side final mixing (exactly mirrors the reference)
# ----------------------------------------------------------------------------


def mix_outputs(y):
    """y: [B, 19] per-event head outputs -> [B, 11] model output."""
    y = y.astype(np.float32)
    morph = y[:, 0:6]
    m = morph - morph.max(axis=1, keepdims=True)
    e = np.exp(m)
    probs = e / e.sum(axis=1, keepdims=True)
    probs = np.maximum(probs, np.float32(1e-6))
    p_cont = probs[:, [0, 1]].sum(1, keepdims=True)
    p_uncont = probs[:, [2, 3, 5]].sum(1, keepdims=True)
    energy = p_cont * y[:, 6:8] + p_uncont * y[:, 8:10]
    p_cas = probs[:, 0:1]
    p_track = probs[:, [1, 2, 3, 5]].sum(1, keepdims=True)
    gate = 1.0 / (1.0 + np.exp(-(energy[:, 0:1] - np.float32(4.0))))
    dirp = p_cas * y[:, 10:13] + p_track * (
        (1.0 - gate) * y[:, 13:16] + gate * y[:, 16:19]
    )
    return np.concatenate([morph, energy, dirp], axis=1).astype(np.float32)


def postprocess(zsums, lay, wp, counts):
    """zsums: [8][19, SLOTS] device outputs -> [B, 11]."""
    y = np.zeros((B, ZD), np.float32)
    ev = lay["ev"]
    slot_len = lay["slot_len"]
    zc = wp["zc"]
    for c in range(N_CORES):
        zs = zsums[c]  # [19, SLOTS]
        e = ev[:, c]
        cnt = counts[e].astype(np.float32)
        pad = (slot_len - counts[e]).astype(np.float32)
        yy = (zs.T - pad[:, None] * zc[None, :]) / np.maximum(cnt, 1.0)[:, None]
        y[e] = yy + wp["bhcat"][None, :]
    return mix_outputs(y)


# ----------------------------------------------------------------------------
# Entry point
# ----------------------------------------------------------------------------

_CACHE = {}
_LAST_RESULT = None  # set when KERNEL_TRACE=1; holds BassKernelResults


def kernel(**inputs):
    import os

    global _LAST_RESULT
    from concourse.bass_utils import run_bass_kernel_spmd

    ins = {k: np.asarray(v) for k, v in inputs.items()}
    coords = ins["coords"].astype(np.float32)
    features = ins["features"].astype(np.float32)
    batch_ids = ins["batch_ids"].astype(np.int64)
    x = np.concatenate([coords, features], axis=1)  # [N, 9]

    counts = np.bincount(batch_ids, minlength=B)
    lay = build_layout(counts)
    wp = pack_weights(ins)
    xall = build_xall(x, batch_ids, lay)

    key = (lay["nw"], tuple(map(tuple, (tuple(w) for w in lay["slots_per_win"]))))
    if key not in _CACHE:
        _CACHE[key] = build_program(lay["nw"], lay["slots_per_win"])
    nc = _CACHE[key]

    shared = {
        k: wp[k] for k in ("w1", "w2a", "w2b", "whbd", "b1", "b2")
    }
    in_maps = [dict(shared, xin=np.ascontiguousarray(xall[c])) for c in range(N_CORES)]
    trace = bool(int(os.environ.get("KERNEL_TRACE", "0")))
    res = run_bass_kernel_spmd(
        nc, in_maps, core_ids=list(range(N_CORES)), trace=trace
    )
    _LAST_RESULT = res
    zsums = [res.results[c]["zsum"] for c in range(N_CORES)]
    return postprocess(zsums, lay, wp, counts)


# revision 11
# speedup vs baseline: 1.1124x; 1.1124x over previous
"""Trainium2 Bass kernel for nn_NeptuneMoEModel (moe_routing).

Model: 6 small MLPs (router + 2 energy experts + 3 direction experts) over
N=262144 points -> segment-mean-pool into B=1024 events -> tiny per-event
head/mixing math.

Strategy (8 NeuronCores, SPMD, data-parallel over events):
  - Events sorted by point count and round-robin assigned to cores so slot s
    on every core holds a similarly-sized event; slot lengths are uniform
    across cores (required: one program for all 8 cores).
  - Slots first-fit packed into 1024-column "windows" (= 2 PSUM banks).
  - Feature-major layout on device: x as [9, S]; layer1 = fused [9, 1536]
    matmul, layer2 = 6x [256,256], all in float32r (1 cyc/row at N=512).
  - All 6 heads fused into one block-diagonal [1536 -> 19] matmul that
    accumulates in PSUM (12 accumulating matmuls per window); pooling then
    reduces only [19, L] per event on the vector engine.
  - gelu (tanh approx, matches jax.nn.gelu) via big [128, 1024] scalar-engine
    activations reading PSUM directly, per-partition bias APs.
  - Host: pad-correction (exact, general for nonzero biases), divide by
    counts, head biases, softmax/gating mixing - all O(B*19) numpy.
"""

import sys

sys.path.insert(0, "/opt/trn_rl_repo")

import numpy as np

import concourse.bass as bass
import concourse.mybir as mybir
import concourse.tile as tile
from concourse import bacc

N_CORES = 8
B = 1024
N_PTS = 262144
DIN = 9
H = 256
NNETS = 6
ZDIMS = [6, 2, 2, 3, 3, 3]
ZOFF = [0, 6, 8, 10, 13, 16]
ZD = 19
WIN = 1024
PIECE = 512
SLOTS = B // N_CORES  # 128
F32 = mybir.dt.float32
BF16 = mybir.dt.bfloat16
try:
    import ml_dtypes

    NPBF16 = ml_dtypes.bfloat16
except ImportError:  # pragma: no cover
    NPBF16 = None
GELU = mybir.ActivationFunctionType.Gelu_apprx_tanh


def _gelu(x):
    """jax.nn.gelu(approximate=True) in numpy/fp32."""
    x = np.asarray(x, np.float32)
    c = np.float32(np.sqrt(2.0 / np.pi))
    return (0.5 * x * (1.0 + np.tanh(c * (x + 0.044715 * x * x * x)))).astype(
        np.float32
    )


# ----------------------------------------------------------------------------
# Layout: event -> (core, slot); slots -> windows
# ----------------------------------------------------------------------------


def build_layout(counts):
    counts = np.asarray(counts)
    order = np.argsort(-counts, kind="stable")
    ev = order.reshape(SLOTS, N_CORES)  # ev[s, c] = event id
    slot_len = counts[ev].max(1)
    slot_len = np.maximum(((slot_len + 3) // 4) * 4, 4).astype(np.int64)
    assert slot_len.max() <= WIN
    # first-fit (slot_len is non-increasing -> this is first-fit-decreasing)
    win_used = []
    slot_win = np.zeros(SLOTS, np.int64)
    slot_off = np.zeros(SLOTS, np.int64)
    for s in range(SLOTS):
        L = int(slot_len[s])
        for w in range(len(win_used)):
            if win_used[w] + L <= WIN:
                slot_win[s] = w
                slot_off[s] = win_used[w]
                win_used[w] += L
                break
        else:
            slot_win[s] = len(win_used)
            slot_off[s] = 0
            win_used.append(L)
    nw = len(win_used)
    slots_per_win = [[] for _ in range(nw)]
    for s in range(SLOTS):
        slots_per_win[slot_win[s]].append(
            (s, int(slot_off[s]), int(slot_len[s]))
        )
    return dict(
        ev=ev,
        slot_len=slot_len,
        slot_win=slot_win,
        slot_off=slot_off,
        nw=nw,
        slots_per_win=slots_per_win,
    )


# ----------------------------------------------------------------------------
# Device program
# ----------------------------------------------------------------------------


def build_program(nw, slots_per_win, slots=SLOTS, act=GELU):
    nc = bacc.Bacc(None, target_bir_lowering=False)
    S = nw * WIN
    xin = nc.dram_tensor("xin", [DIN, S], BF16, kind="ExternalInput")
    w1 = nc.dram_tensor("w1", [DIN, 12 * 128], BF16, kind="ExternalInput")
    w2a = nc.dram_tensor("w2a", [128, NNETS * 256], BF16, kind="ExternalInput")
    w2b = nc.dram_tensor("w2b", [128, NNETS * 256], BF16, kind="ExternalInput")
    whbd = nc.dram_tensor("whbd", [128, 12 * ZD], BF16, kind="ExternalInput")
    b1 = nc.dram_tensor("b1", [128, 12], F32, kind="ExternalInput")
    b2 = nc.dram_tensor("b2", [128, 12], F32, kind="ExternalInput")
    outt = nc.dram_tensor("zsum", [ZD, 2 * slots], F32, kind="ExternalOutput")

    npiece = WIN // PIECE
    with tile.TileContext(nc) as tc:
        with (
            tc.tile_pool(name="wts", bufs=1) as wts,
            tc.tile_pool(name="xp", bufs=3) as xp,
            tc.tile_pool(name="h1p", bufs=14) as h1p,
            tc.tile_pool(name="h2p", bufs=4) as h2p,
            tc.tile_pool(name="op", bufs=1) as op,
            tc.tile_pool(name="psm", bufs=3, space="PSUM") as psm,
            tc.tile_pool(name="psz", bufs=2, space="PSUM") as psz,
        ):
            w1t = wts.tile([DIN, 12 * 128], BF16)
            nc.sync.dma_start(w1t, w1[:, :])
            w2t = [wts.tile([128, NNETS * 256], BF16, name=f"w2_{k}") for k in range(2)]
            nc.sync.dma_start(w2t[0], w2a[:, :])
            nc.sync.dma_start(w2t[1], w2b[:, :])
            whbdt = wts.tile([128, 12 * ZD], BF16)
            nc.sync.dma_start(whbdt, whbd[:, :])
            b1t = wts.tile([128, 12], F32)
            nc.sync.dma_start(b1t, b1[:, :])
            b2t = wts.tile([128, 12], F32)
            nc.sync.dma_start(b2t, b2[:, :])
            zsb = op.tile([ZD, 2 * slots], F32)
            nc.vector.memset(zsb, 0.0)

            for w in range(nw):
                xw = xp.tile([DIN, WIN], BF16, tag="xw")
                nc.sync.dma_start(xw, xin[:, w * WIN : (w + 1) * WIN])
                # layer 1: h1[j] = gelu(x @ W1[:, j-tile] + b1), j = net*2 + half
                h1 = []
                for j in range(12):
                    ps = psm.tile([128, WIN], F32, tag="ps")
                    for p in range(npiece):
                        sl = slice(p * PIECE, (p + 1) * PIECE)
                        nc.tensor.matmul(
                            ps[:, sl],
                            w1t[:, j * 128 : (j + 1) * 128],
                            xw[:, sl],
                            start=True,
                            stop=True,
                        )
                    t = h1p.tile([128, WIN], BF16, tag="h1")
                    nc.scalar.activation(t, ps, act, bias=b1t[:, j : j + 1])
                    h1.append(t)
                # layer 2 + block-diagonal head accumulated into z psum
                # (one [19, 512] psum tile per piece: 1 bank each)
                z = [
                    psz.tile([ZD, PIECE], F32, tag="z", name=f"z{w}_{p}")
                    for p in range(npiece)
                ]
                for n in range(NNETS):
                    h2 = []
                    for mo in range(2):
                        j = 2 * n + mo
                        ps = psm.tile([128, WIN], F32, tag="ps")
                        c0 = n * 256 + mo * 128
                        for k in range(2):
                            for p in range(npiece):
                                sl = slice(p * PIECE, (p + 1) * PIECE)
                                nc.tensor.matmul(
                                    ps[:, sl],
                                    w2t[k][:, c0 : c0 + 128],
                                    h1[2 * n + k][:, sl],
                                    start=(k == 0),
                                    stop=(k == 1),
                                    skip_group_check=True,
                                )
                        t = h2p.tile([128, WIN], BF16, tag="h2")
                        nc.scalar.activation(t, ps, act, bias=b2t[:, j : j + 1])
                        h2.append(t)
                    for mo in range(2):
                        j = 2 * n + mo
                        for p in range(npiece):
                            sl = slice(p * PIECE, (p + 1) * PIECE)
                            nc.tensor.matmul(
                                z[p][:, :],
                                whbdt[:, j * ZD : (j + 1) * ZD],
                                h2[mo][:, sl],
                                start=(j == 0),
                                stop=(j == 11),
                                skip_group_check=True,
                            )
                # per-event pooling: sum z columns of each slot; a slot may
                # straddle the two pieces -> two partials, host adds them
                for s, off, L in slots_per_win[w]:
                    q0 = off // PIECE
                    q1 = (off + L - 1) // PIECE
                    if q0 == q1:
                        nc.vector.tensor_reduce(
                            zsb[:, s : s + 1],
                            z[q0][:, off - q0 * PIECE : off - q0 * PIECE + L],
                            axis=mybir.AxisListType.X,
                            op=mybir.AluOpType.add,
                        )
                    else:
                        nc.vector.tensor_reduce(
                            zsb[:, s : s + 1],
                            z[q0][:, off - q0 * PIECE :],
                            axis=mybir.AxisListType.X,
                            op=mybir.AluOpType.add,
                        )
                        nc.vector.tensor_reduce(
                            zsb[:, slots + s : slots + s + 1],
                            z[q1][:, : off + L - q1 * PIECE],
                            axis=mybir.AxisListType.X,
                            op=mybir.AluOpType.add,
                        )
            nc.sync.dma_start(outt[:, :], zsb)
    nc.compile()
    return nc


# ----------------------------------------------------------------------------
# Host-side weight packing
# ----------------------------------------------------------------------------


def pack_weights(ins):
    W1s = [ins["router_W1"]] + [ins["e_W1"][i] for i in range(2)] + [
        ins["d_W1"][i] for i in range(3)
    ]
    W2s = [ins["router_W2"]] + [ins["e_W2"][i] for i in range(2)] + [
        ins["d_W2"][i] for i in range(3)
    ]
    Whs = [ins["router_Wh"]] + [ins["e_Wh"][i] for i in range(2)] + [
        ins["d_Wh"][i] for i in range(3)
    ]
    b1s = [ins["router_b1"]] + [ins["e_b1"][i] for i in range(2)] + [
        ins["d_b1"][i] for i in range(3)
    ]
    b2s = [ins["router_b2"]] + [ins["e_b2"][i] for i in range(2)] + [
        ins["d_b2"][i] for i in range(3)
    ]
    bhs = [ins["router_bh"]] + [ins["e_bh"][i] for i in range(2)] + [
        ins["d_bh"][i] for i in range(3)
    ]
    f = lambda a: np.ascontiguousarray(np.asarray(a, np.float32))
    W1cat = np.concatenate([f(w) for w in W1s], axis=1)  # [9, 1536]
    w2a = np.concatenate([f(w)[0:128, :] for w in W2s], axis=1)  # [128, 1536]
    w2b = np.concatenate([f(w)[128:256, :] for w in W2s], axis=1)
    b1cat = np.concatenate([f(b) for b in b1s])  # [1536]
    b2cat = np.concatenate([f(b) for b in b2s])
    bhcat = np.concatenate([f(b) for b in bhs])  # [19]
    whbd = np.zeros((128, 12 * ZD), np.float32)
    for j in range(12):
        nt, hj = j // 2, j % 2
        whbd[:, j * ZD + ZOFF[nt] : j * ZD + ZOFF[nt] + ZDIMS[nt]] = f(Whs[nt])[
            hj * 128 : (hj + 1) * 128, :
        ]
    b1t = b1cat.reshape(12, 128).T.copy()  # [128, 12]
    b2t = b2cat.reshape(12, 128).T.copy()
    # pad-column contribution (exact; zero when biases are zero)
    h1c = _gelu(b1cat)
    zc = np.zeros(ZD, np.float32)
    for n in range(NNETS):
        a2c = h1c[n * 256 : (n + 1) * 256] @ f(W2s[n]) + f(b2s[n])
        h2c = _gelu(a2c)
        zc[ZOFF[n] : ZOFF[n] + ZDIMS[n]] = h2c @ f(Whs[n])
    bf = lambda a: a.astype(NPBF16)
    return dict(
        w1=bf(W1cat), w2a=bf(w2a), w2b=bf(w2b), whbd=bf(whbd), b1=b1t, b2=b2t,
        bhcat=bhcat,
        zc=zc, W2s=[f(w) for w in W2s], Whs=[f(w) for w in Whs],
        b2s=[f(b) for b in b2s],
    )


def build_xall(x, batch_ids, lay):
    """Scatter points into per-core feature-major padded streams [8, 9, S]."""
    counts = np.bincount(batch_ids, minlength=B)
    seg_start = np.zeros(B, np.int64)
    np.cumsum(counts[:-1], out=seg_start[1:])
    rank = np.empty(B, np.int64)
    rank[lay["ev"].reshape(-1)] = np.arange(B)
    r = rank[batch_ids]
    s = r // N_CORES
    c = r % N_CORES
    pos = np.arange(N_PTS) - seg_start[batch_ids]
    col = lay["slot_win"][s] * WIN + lay["slot_off"][s] + pos
    S = lay["nw"] * WIN
    xall = np.zeros((N_CORES, DIN, S), NPBF16)
    xall[c, :, col] = x.astype(NPBF16)
    return xall


# ----------------------------------------------------------------------------
# Host-side final mixing (exactly mirrors the reference)
# ----------------------------------------------------------------------------


def mix_outputs(y):
    """y: [B, 19] per-event head outputs -> [B, 11] model output."""
    y = y.astype(np.float32)
    morph = y[:, 0:6]
    m = morph - morph.max(axis=1, keepdims=True)
    e = np.exp(m)
    probs = e / e.sum(axis=1, keepdims=True)
    probs = np.maximum(probs, np.float32(1e-6))
    p_cont = probs[:, [0, 1]].sum(1, keepdims=True)
    p_uncont = probs[:, [2, 3, 5]].sum(1, keepdims=True)
    energy = p_cont * y[:, 6:8] + p_uncont * y[:, 8:10]
    p_cas = probs[:, 0:1]
    p_track = probs[:, [1, 2, 3, 5]].sum(1, keepdims=True)
    gate = 1.0 / (1.0 + np.exp(-(energy[:, 0:1] - np.float32(4.0))))
    dirp = p_cas * y[:, 10:13] + p_track * (
        (1.0 - gate) * y[:, 13:16] + gate * y[:, 16:19]
    )
    return np.concatenate([morph, energy, dirp], axis=1).astype(np.float32)


def postprocess(zsums, lay, wp, counts):
    """zsums: [8][19, SLOTS] device outputs -> [B, 11]."""
    y = np.zeros((B, ZD), np.float32)
    ev = lay["ev"]
    slot_len = lay["slot_len"]
    zc = wp["zc"]
    for c in range(N_CORES):
        zs = zsums[c]
        zs = zs[:, :SLOTS] + zs[:, SLOTS:]  # combine straddle partials
        e = ev[:, c]
        cnt = counts[e].astype(np.float32)
        pad = (slot_len - counts[e]).astype(np.float32)
        yy = (zs.T - pad[:, None] * zc[None, :]) / np.maximum(cnt, 1.0)[:, None]
        y[e] = yy + wp["bhcat"][None, :]
    return mix_outputs(y)


# ----------------------------------------------------------------------------
# Entry point
# ----------------------------------------------------------------------------

_CACHE = {}
_LAST_RESULT = None  # set when KERNEL_TRACE=1; holds BassKernelResults


def kernel(**inputs):
    import os

    global _LAST_RESULT
    from concourse.bass_utils import run_bass_kernel_spmd

    ins = {k: np.asarray(v) for k, v in inputs.items()}
    coords = ins["coords"].astype(np.float32)
    features = ins["features"].astype(np.float32)
    batch_ids = ins["batch_ids"].astype(np.int64)
    x = np.concatenate([coords, features], axis=1)  # [N, 9]

    counts = np.bincount(batch_ids, minlength=B)
    lay = build_layout(counts)
    wp = pack_weights(ins)
    xall = build_xall(x, batch_ids, lay)

    key = (lay["nw"], tuple(map(tuple, (tuple(w) for w in lay["slots_per_win"]))))
    if key not in _CACHE:
        _CACHE[key] = build_program(lay["nw"], lay["slots_per_win"])
    nc = _CACHE[key]

    shared = {
        k: wp[k] for k in ("w1", "w2a", "w2b", "whbd", "b1", "b2")
    }
    in_maps = [dict(shared, xin=np.ascontiguousarray(xall[c])) for c in range(N_CORES)]
    trace = bool(int(os.environ.get("KERNEL_TRACE", "0")))
    res = run_bass_kernel_spmd(
        nc, in_maps, core_ids=list(range(N_CORES)), trace=trace
    )
    _LAST_RESULT = res
    zsums = [res.results[c]["zsum"] for c in range(N_CORES)]
    return postprocess(zsums, lay, wp, counts)


# revision 13
# speedup vs baseline: 1.1862x; 1.0663x over previous
"""Trainium2 Bass kernel for nn_NeptuneMoEModel (moe_routing).

Model: 6 small MLPs (router + 2 energy experts + 3 direction experts) over
N=262144 points -> segment-mean-pool into B=1024 events -> tiny per-event
head/mixing math.

Strategy (8 NeuronCores, SPMD, data-parallel over events):
  - Events sorted by point count and round-robin assigned to cores so slot s
    on every core holds a similarly-sized event; slot lengths are uniform
    across cores (required: one program for all 8 cores).
  - Slots first-fit packed into 1024-column "windows" (= 2 PSUM banks).
  - Feature-major layout on device: x as [9, S]; layer1 = fused [9, 1536]
    matmul, layer2 = 6x [256,256], all in float32r (1 cyc/row at N=512).
  - All 6 heads fused into one block-diagonal [1536 -> 19] matmul that
    accumulates in PSUM (12 accumulating matmuls per window); pooling then
    reduces only [19, L] per event on the vector engine.
  - gelu (tanh approx, matches jax.nn.gelu) via big [128, 1024] scalar-engine
    activations reading PSUM directly, per-partition bias APs.
  - Host: pad-correction (exact, general for nonzero biases), divide by
    counts, head biases, softmax/gating mixing - all O(B*19) numpy.
"""

import sys

sys.path.insert(0, "/opt/trn_rl_repo")

import numpy as np

import concourse.bass as bass
import concourse.mybir as mybir
import concourse.tile as tile
from concourse import bacc

N_CORES = 8
B = 1024
N_PTS = 262144
DIN = 9
H = 256
NNETS = 6
ZDIMS = [6, 2, 2, 3, 3, 3]
ZOFF = [0, 6, 8, 10, 13, 16]
ZD = 19
WIN = 1024
PIECE = 512
SLOTS = B // N_CORES  # 128
F32 = mybir.dt.float32
BF16 = mybir.dt.bfloat16
try:
    import ml_dtypes

    NPBF16 = ml_dtypes.bfloat16
except ImportError:  # pragma: no cover
    NPBF16 = None
GELU = mybir.ActivationFunctionType.Gelu_apprx_tanh


def _gelu(x):
    """jax.nn.gelu(approximate=True) in numpy/fp32."""
    x = np.asarray(x, np.float32)
    c = np.float32(np.sqrt(2.0 / np.pi))
    return (0.5 * x * (1.0 + np.tanh(c * (x + 0.044715 * x * x * x)))).astype(
        np.float32
    )


# ----------------------------------------------------------------------------
# Layout: event -> (core, slot); slots -> windows
# ----------------------------------------------------------------------------


def build_layout(counts):
    counts = np.asarray(counts)
    order = np.argsort(-counts, kind="stable")
    ev = order.reshape(SLOTS, N_CORES)  # ev[s, c] = event id
    slot_len = counts[ev].max(1)
    slot_len = np.maximum(((slot_len + 3) // 4) * 4, 4).astype(np.int64)
    assert slot_len.max() <= WIN
    # first-fit (slot_len is non-increasing -> this is first-fit-decreasing)
    win_used = []
    slot_win = np.zeros(SLOTS, np.int64)
    slot_off = np.zeros(SLOTS, np.int64)
    for s in range(SLOTS):
        L = int(slot_len[s])
        for w in range(len(win_used)):
            if win_used[w] + L <= WIN:
                slot_win[s] = w
                slot_off[s] = win_used[w]
                win_used[w] += L
                break
        else:
            slot_win[s] = len(win_used)
            slot_off[s] = 0
            win_used.append(L)
    nw = len(win_used)
    slots_per_win = [[] for _ in range(nw)]
    for s in range(SLOTS):
        slots_per_win[slot_win[s]].append(
            (s, int(slot_off[s]), int(slot_len[s]))
        )
    win_cols = [min(WIN, ((u + 7) // 8) * 8) for u in win_used]
    return dict(
        ev=ev,
        slot_len=slot_len,
        slot_win=slot_win,
        slot_off=slot_off,
        nw=nw,
        slots_per_win=slots_per_win,
        win_cols=win_cols,
    )


# ----------------------------------------------------------------------------
# Device program
# ----------------------------------------------------------------------------


def build_program(nw, slots_per_win, win_cols=None, slots=SLOTS, act=GELU):
    """L1 row-packed 3x (K=9 strips at partitions 0/32/64), L2 plain,
    block-diagonal heads col-packed 4x into [115, 512] z psum tiles."""
    nc = bacc.Bacc(None, target_bir_lowering=False)
    if win_cols is None:
        win_cols = [WIN] * nw
    ZROWS = 96 + ZD  # 4 col strips at partitions 0/32/64/96
    S = nw * WIN
    xin = nc.dram_tensor("xin", [DIN, S], BF16, kind="ExternalInput")
    w1 = nc.dram_tensor("w1", [73, 512], BF16, kind="ExternalInput")
    w2a = nc.dram_tensor("w2a", [128, NNETS * 256], BF16, kind="ExternalInput")
    w2b = nc.dram_tensor("w2b", [128, NNETS * 256], BF16, kind="ExternalInput")
    whbd = nc.dram_tensor("whbd", [128, 12 * ZD], BF16, kind="ExternalInput")
    b1 = nc.dram_tensor("b1", [128, 12], F32, kind="ExternalInput")
    b2 = nc.dram_tensor("b2", [128, 12], F32, kind="ExternalInput")
    outt = nc.dram_tensor("zsum", [ZROWS, 2 * slots], F32, kind="ExternalOutput")

    with tile.TileContext(nc) as tc:
        with (
            tc.tile_pool(name="wts", bufs=1) as wts,
            tc.tile_pool(name="xp", bufs=3) as xp,
            tc.tile_pool(name="h1p", bufs=14) as h1p,
            tc.tile_pool(name="h2p", bufs=4) as h2p,
            tc.tile_pool(name="op", bufs=1) as op,
            tc.tile_pool(name="psm", bufs=3, space="PSUM") as psm,
            tc.tile_pool(name="psz", bufs=2, space="PSUM") as psz,
        ):
            w1t = wts.tile([73, 512], BF16)
            nc.sync.dma_start(w1t, w1[:, :])
            w2t = [wts.tile([128, NNETS * 256], BF16, name=f"w2_{k}") for k in range(2)]
            nc.sync.dma_start(w2t[0], w2a[:, :])
            nc.sync.dma_start(w2t[1], w2b[:, :])
            whbdt = wts.tile([128, 12 * ZD], BF16)
            nc.sync.dma_start(whbdt, whbd[:, :])
            b1t = wts.tile([128, 12], F32)
            nc.sync.dma_start(b1t, b1[:, :])
            b2t = wts.tile([128, 12], F32)
            nc.sync.dma_start(b2t, b2[:, :])
            zsb = op.tile([ZROWS, 2 * slots], F32)
            nc.vector.memset(zsb, 0.0)

            for w in range(nw):
                used = win_cols[w]
                pieces = [(p, min(p + PIECE, used)) for p in range(0, used, PIECE)]
                xw = xp.tile([73, WIN], BF16, tag="xw")
                for r in range(3):
                    nc.sync.dma_start(
                        xw[32 * r : 32 * r + DIN, :used],
                        xin[:, w * WIN : w * WIN + used],
                    )
                # layer 1 row-packed: strips r=0..2 run concurrently on PE
                h1 = [None] * 12
                for g in range(4):
                    pss = []
                    for r in range(3):
                        j = 3 * g + r
                        ps = psm.tile([128, WIN], F32, tag="ps", name=f"ps1_{w}_{j}")
                        for a, b in pieces:
                            nc.tensor.matmul(
                                ps[:, a:b],
                                w1t[32 * r : 32 * r + DIN, 128 * g : 128 * (g + 1)],
                                xw[32 * r : 32 * r + DIN, a:b],
                                start=True,
                                stop=True,
                                tile_position=(32 * r, 0),
                            )
                        pss.append(ps)
                    for r in range(3):
                        j = 3 * g + r
                        t = h1p.tile([128, WIN], BF16, tag="h1", name=f"h1_{w}_{j}")
                        nc.scalar.activation(
                            t[:, :used], pss[r][:, :used], act, bias=b1t[:, j : j + 1]
                        )
                        h1[j] = t
                # layer 2 + col-packed block-diagonal heads into z psum
                z = [
                    psz.tile([ZROWS, PIECE], F32, tag="z", name=f"z{w}_{q}")
                    for q in range(len(pieces))
                ]
                for q in range(len(pieces)):
                    nc.vector.memset(z[q], 0.0)
                for n in range(NNETS):
                    h2 = []
                    for mo in range(2):
                        j = 2 * n + mo
                        ps = psm.tile([128, WIN], F32, tag="ps", name=f"ps2_{w}_{j}")
                        c0 = n * 256 + mo * 128
                        for k in range(2):
                            for a, b in pieces:
                                nc.tensor.matmul(
                                    ps[:, a:b],
                                    w2t[k][:, c0 : c0 + 128],
                                    h1[2 * n + k][:, a:b],
                                    start=(k == 0),
                                    stop=(k == 1),
                                    skip_group_check=True,
                                )
                        t = h2p.tile([128, WIN], BF16, tag="h2", name=f"h2_{w}_{j}")
                        nc.scalar.activation(
                            t[:, :used], ps[:, :used], act, bias=b2t[:, j : j + 1]
                        )
                        h2.append(t)
                    for mo in range(2):
                        j = 2 * n + mo
                        st = 32 * (j % 4)
                        for q, (a, b) in enumerate(pieces):
                            nc.tensor.matmul(
                                z[q][st : st + ZD, : b - a],
                                whbdt[:, j * ZD : (j + 1) * ZD],
                                h2[mo][:, a:b],
                                start=(j < 4),
                                stop=(j >= 8),
                                tile_position=(0, st),
                                skip_group_check=True,
                            )
                # per-event pooling: sum z columns of each slot; a slot may
                # straddle the two pieces -> two partials, host adds them
                for s, off, L in slots_per_win[w]:
                    q0 = off // PIECE
                    q1 = (off + L - 1) // PIECE
                    if q0 == q1:
                        nc.vector.tensor_reduce(
                            zsb[:, s : s + 1],
                            z[q0][:, off - q0 * PIECE : off - q0 * PIECE + L],
                            axis=mybir.AxisListType.X,
                            op=mybir.AluOpType.add,
                        )
                    else:
                        nc.vector.tensor_reduce(
                            zsb[:, s : s + 1],
                            z[q0][:, off - q0 * PIECE :],
                            axis=mybir.AxisListType.X,
                            op=mybir.AluOpType.add,
                        )
                        nc.vector.tensor_reduce(
                            zsb[:, slots + s : slots + s + 1],
                            z[q1][:, : off + L - q1 * PIECE],
                            axis=mybir.AxisListType.X,
                            op=mybir.AluOpType.add,
                        )
            nc.sync.dma_start(outt[:, :], zsb)
    nc.compile()
    return nc


# ----------------------------------------------------------------------------
# Host-side weight packing
# ----------------------------------------------------------------------------


def pack_weights(ins):
    W1s = [ins["router_W1"]] + [ins["e_W1"][i] for i in range(2)] + [
        ins["d_W1"][i] for i in range(3)
    ]
    W2s = [ins["router_W2"]] + [ins["e_W2"][i] for i in range(2)] + [
        ins["d_W2"][i] for i in range(3)
    ]
    Whs = [ins["router_Wh"]] + [ins["e_Wh"][i] for i in range(2)] + [
        ins["d_Wh"][i] for i in range(3)
    ]
    b1s = [ins["router_b1"]] + [ins["e_b1"][i] for i in range(2)] + [
        ins["d_b1"][i] for i in range(3)
    ]
    b2s = [ins["router_b2"]] + [ins["e_b2"][i] for i in range(2)] + [
        ins["d_b2"][i] for i in range(3)
    ]
    bhs = [ins["router_bh"]] + [ins["e_bh"][i] for i in range(2)] + [
        ins["d_bh"][i] for i in range(3)
    ]
    f = lambda a: np.ascontiguousarray(np.asarray(a, np.float32))
    W1cat = np.concatenate([f(w) for w in W1s], axis=1)  # [9, 1536]
    W1stack = np.zeros((73, 512), np.float32)
    for j in range(12):
        g, r = j // 3, j % 3
        W1stack[32 * r : 32 * r + DIN, 128 * g : 128 * (g + 1)] = W1cat[
            :, 128 * j : 128 * (j + 1)
        ]
    w2a = np.concatenate([f(w)[0:128, :] for w in W2s], axis=1)  # [128, 1536]
    w2b = np.concatenate([f(w)[128:256, :] for w in W2s], axis=1)
    b1cat = np.concatenate([f(b) for b in b1s])  # [1536]
    b2cat = np.concatenate([f(b) for b in b2s])
    bhcat = np.concatenate([f(b) for b in bhs])  # [19]
    whbd = np.zeros((128, 12 * ZD), np.float32)
    for j in range(12):
        nt, hj = j // 2, j % 2
        whbd[:, j * ZD + ZOFF[nt] : j * ZD + ZOFF[nt] + ZDIMS[nt]] = f(Whs[nt])[
            hj * 128 : (hj + 1) * 128, :
        ]
    b1t = b1cat.reshape(12, 128).T.copy()  # [128, 12]
    b2t = b2cat.reshape(12, 128).T.copy()
    # pad-column contribution (exact; zero when biases are zero)
    h1c = _gelu(b1cat)
    zc = np.zeros(ZD, np.float32)
    for n in range(NNETS):
        a2c = h1c[n * 256 : (n + 1) * 256] @ f(W2s[n]) + f(b2s[n])
        h2c = _gelu(a2c)
        zc[ZOFF[n] : ZOFF[n] + ZDIMS[n]] = h2c @ f(Whs[n])
    bf = lambda a: a.astype(NPBF16)
    return dict(
        w1=bf(W1stack), w2a=bf(w2a), w2b=bf(w2b), whbd=bf(whbd), b1=b1t, b2=b2t,
        bhcat=bhcat,
        zc=zc, W2s=[f(w) for w in W2s], Whs=[f(w) for w in Whs],
        b2s=[f(b) for b in b2s],
    )


def build_xall(x, batch_ids, lay):
    """Scatter points into per-core feature-major padded streams [8, 9, S]."""
    counts = np.bincount(batch_ids, minlength=B)
    seg_start = np.zeros(B, np.int64)
    np.cumsum(counts[:-1], out=seg_start[1:])
    rank = np.empty(B, np.int64)
    rank[lay["ev"].reshape(-1)] = np.arange(B)
    r = rank[batch_ids]
    s = r // N_CORES
    c = r % N_CORES
    pos = np.arange(N_PTS) - seg_start[batch_ids]
    col = lay["slot_win"][s] * WIN + lay["slot_off"][s] + pos
    S = lay["nw"] * WIN
    xall = np.zeros((N_CORES, DIN, S), NPBF16)
    xall[c, :, col] = x.astype(NPBF16)
    return xall


# ----------------------------------------------------------------------------
# Host-side final mixing (exactly mirrors the reference)
# ----------------------------------------------------------------------------


def mix_outputs(y):
    """y: [B, 19] per-event head outputs -> [B, 11] model output."""
    y = y.astype(np.float32)
    morph = y[:, 0:6]
    m = morph - morph.max(axis=1, keepdims=True)
    e = np.exp(m)
    probs = e / e.sum(axis=1, keepdims=True)
    probs = np.maximum(probs, np.float32(1e-6))
    p_cont = probs[:, [0, 1]].sum(1, keepdims=True)
    p_uncont = probs[:, [2, 3, 5]].sum(1, keepdims=True)
    energy = p_cont * y[:, 6:8] + p_uncont * y[:, 8:10]
    p_cas = probs[:, 0:1]
    p_track = probs[:, [1, 2, 3, 5]].sum(1, keepdims=True)
    gate = 1.0 / (1.0 + np.exp(-(energy[:, 0:1] - np.float32(4.0))))
    dirp = p_cas * y[:, 10:13] + p_track * (
        (1.0 - gate) * y[:, 13:16] + gate * y[:, 16:19]
    )
    return np.concatenate([morph, energy, dirp], axis=1).astype(np.float32)


def postprocess(zsums, lay, wp, counts):
    """zsums: [8][19, SLOTS] device outputs -> [B, 11]."""
    y = np.zeros((B, ZD), np.float32)
    ev = lay["ev"]
    slot_len = lay["slot_len"]
    zc = wp["zc"]
    for c in range(N_CORES):
        zf = zsums[c]  # [115, 2*SLOTS]
        zs = np.zeros((ZD, SLOTS), np.float32)
        for st in range(4):
            zs += zf[32 * st : 32 * st + ZD, :SLOTS] + zf[32 * st : 32 * st + ZD, SLOTS:]
        e = ev[:, c]
        cnt = counts[e].astype(np.float32)
        pad = (slot_len - counts[e]).astype(np.float32)
        yy = (zs.T - pad[:, None] * zc[None, :]) / np.maximum(cnt, 1.0)[:, None]
        y[e] = yy + wp["bhcat"][None, :]
    return mix_outputs(y)


# ----------------------------------------------------------------------------
# Entry point
# ----------------------------------------------------------------------------

_CACHE = {}
_LAST_RESULT = None  # set when KERNEL_TRACE=1; holds BassKernelResults


def kernel(**inputs):
    import os

    global _LAST_RESULT
    from concourse.bass_utils import run_bass_kernel_spmd

    ins = {k: np.asarray(v) for k, v in inputs.items()}
    coords = ins["coords"].astype(np.float32)
    features = ins["features"].astype(np.float32)
    batch_ids = ins["batch_ids"].astype(np.int64)
    x = np.concatenate([coords, features], axis=1)  # [N, 9]

    counts = np.bincount(batch_ids, minlength=B)
    lay = build_layout(counts)
    wp = pack_weights(ins)
    xall = build_xall(x, batch_ids, lay)

    key = (lay["nw"], tuple(map(tuple, (tuple(w) for w in lay["slots_per_win"]))))
    key = (key, tuple(lay["win_cols"]))
    if key not in _CACHE:
        _CACHE[key] = build_program(
            lay["nw"], lay["slots_per_win"], win_cols=lay["win_cols"]
        )
    nc = _CACHE[key]

    shared = {
        k: wp[k] for k in ("w1", "w2a", "w2b", "whbd", "b1", "b2")
    }
    in_maps = [dict(shared, xin=np.ascontiguousarray(xall[c])) for c in range(N_CORES)]
    trace = bool(int(os.environ.get("KERNEL_TRACE", "0")))
    res = run_bass_kernel_spmd(
        nc, in_maps, core_ids=list(range(N_CORES)), trace=trace
    )
    _LAST_RESULT = res
    zsums = [res.results[c]["zsum"] for c in range(N_CORES)]
    return postprocess(zsums, lay, wp, counts)


# revision 16
# speedup vs baseline: 1.3094x; 1.1038x over previous
"""Trainium2 Bass kernel for nn_NeptuneMoEModel (moe_routing).

Model: 6 small MLPs (router + 2 energy experts + 3 direction experts) over
N=262144 points -> segment-mean-pool into B=1024 events -> tiny per-event
head/mixing math.

Strategy (8 NeuronCores, SPMD, data-parallel over events):
  - Events sorted by point count and round-robin assigned to cores so slot s
    on every core holds a similarly-sized event; slot lengths are uniform
    across cores (required: one program for all 8 cores).
  - Slots first-fit packed into 1024-column "windows" (= 2 PSUM banks).
  - Feature-major layout on device: x as [9, S]; layer1 = fused [9, 1536]
    matmul, layer2 = 6x [256,256], all in float32r (1 cyc/row at N=512).
  - All 6 heads fused into one block-diagonal [1536 -> 19] matmul that
    accumulates in PSUM (12 accumulating matmuls per window); pooling then
    reduces only [19, L] per event on the vector engine.
  - gelu (tanh approx, matches jax.nn.gelu) via big [128, 1024] scalar-engine
    activations reading PSUM directly, per-partition bias APs.
  - Host: pad-correction (exact, general for nonzero biases), divide by
    counts, head biases, softmax/gating mixing - all O(B*19) numpy.
"""

import sys

sys.path.insert(0, "/opt/trn_rl_repo")

import numpy as np

import concourse.bass as bass
import concourse.mybir as mybir
import concourse.tile as tile
from concourse import bacc

N_CORES = 8
B = 1024
N_PTS = 262144
DIN = 9
H = 256
NNETS = 6
ZDIMS = [6, 2, 2, 3, 3, 3]
ZOFF = [0, 6, 8, 10, 13, 16]
ZD = 19
WIN = 1024
PIECE = 512
SLOTS = B // N_CORES  # 128
F32 = mybir.dt.float32
BF16 = mybir.dt.bfloat16
try:
    import ml_dtypes

    NPBF16 = ml_dtypes.bfloat16
except ImportError:  # pragma: no cover
    NPBF16 = None
GELU = mybir.ActivationFunctionType.Gelu_apprx_tanh


def _gelu(x):
    """jax.nn.gelu(approximate=True) in numpy/fp32."""
    x = np.asarray(x, np.float32)
    c = np.float32(np.sqrt(2.0 / np.pi))
    return (0.5 * x * (1.0 + np.tanh(c * (x + 0.044715 * x * x * x)))).astype(
        np.float32
    )


# ----------------------------------------------------------------------------
# Layout: event -> (core, slot); slots -> windows
# ----------------------------------------------------------------------------


def build_layout(counts):
    counts = np.asarray(counts)
    order = np.argsort(-counts, kind="stable")
    ev = order.reshape(SLOTS, N_CORES)  # ev[s, c] = event id
    slot_len = counts[ev].max(1)
    slot_len = np.maximum(((slot_len + 3) // 4) * 4, 4).astype(np.int64)
    assert slot_len.max() <= WIN
    # first-fit (slot_len is non-increasing -> this is first-fit-decreasing)
    win_used = []
    slot_win = np.zeros(SLOTS, np.int64)
    slot_off = np.zeros(SLOTS, np.int64)
    for s in range(SLOTS):
        L = int(slot_len[s])
        for w in range(len(win_used)):
            if win_used[w] + L <= WIN:
                slot_win[s] = w
                slot_off[s] = win_used[w]
                win_used[w] += L
                break
        else:
            slot_win[s] = len(win_used)
            slot_off[s] = 0
            win_used.append(L)
    nw = len(win_used)
    slots_per_win = [[] for _ in range(nw)]
    for s in range(SLOTS):
        slots_per_win[slot_win[s]].append(
            (s, int(slot_off[s]), int(slot_len[s]))
        )
    win_cols = [min(WIN, ((u + 7) // 8) * 8) for u in win_used]
    return dict(
        ev=ev,
        slot_len=slot_len,
        slot_win=slot_win,
        slot_off=slot_off,
        nw=nw,
        slots_per_win=slots_per_win,
        win_cols=win_cols,
    )


# ----------------------------------------------------------------------------
# Device program
# ----------------------------------------------------------------------------


def build_program(nw, slots_per_win, win_cols=None, slots=SLOTS, act=GELU):
    """L1 row-packed 3x (K=9 strips at partitions 0/32/64), L2 plain,
    block-diagonal heads col-packed 4x into [115, 512] z psum tiles."""
    nc = bacc.Bacc(None, target_bir_lowering=False)
    if win_cols is None:
        win_cols = [WIN] * nw
    ZROWS = 96 + ZD  # 4 col strips at partitions 0/32/64/96
    S = nw * WIN
    xin = nc.dram_tensor("xin", [DIN, S], BF16, kind="ExternalInput")
    w1 = nc.dram_tensor("w1", [73, 512], BF16, kind="ExternalInput")
    w2a = nc.dram_tensor("w2a", [128, NNETS * 256], BF16, kind="ExternalInput")
    w2b = nc.dram_tensor("w2b", [128, NNETS * 256], BF16, kind="ExternalInput")
    whbd = nc.dram_tensor("whbd", [128, 12 * ZD], BF16, kind="ExternalInput")
    b1 = nc.dram_tensor("b1", [128, 12], F32, kind="ExternalInput")
    b2 = nc.dram_tensor("b2", [128, 12], F32, kind="ExternalInput")
    outt = nc.dram_tensor("zsum", [ZROWS, 2 * slots], F32, kind="ExternalOutput")

    with tile.TileContext(nc) as tc:
        with (
            tc.tile_pool(name="wts", bufs=1) as wts,
            tc.tile_pool(name="xp", bufs=3) as xp,
            tc.tile_pool(name="h1p", bufs=14) as h1p,
            tc.tile_pool(name="h2p", bufs=26) as h2p,
            tc.tile_pool(name="op", bufs=1) as op,
            tc.tile_pool(name="psm", bufs=3, space="PSUM") as psm,
            tc.tile_pool(name="psz", bufs=2, space="PSUM") as psz,
        ):
            w1t = wts.tile([73, 512], BF16)
            nc.sync.dma_start(w1t, w1[:, :])
            w2t = [wts.tile([128, NNETS * 256], BF16, name=f"w2_{k}") for k in range(2)]
            nc.sync.dma_start(w2t[0], w2a[:, :])
            nc.sync.dma_start(w2t[1], w2b[:, :])
            whbdt = wts.tile([128, 12 * ZD], BF16)
            nc.sync.dma_start(whbdt, whbd[:, :])
            b1t = wts.tile([128, 12], F32)
            nc.sync.dma_start(b1t, b1[:, :])
            b2t = wts.tile([128, 12], F32)
            nc.sync.dma_start(b2t, b2[:, :])
            zsb = op.tile([ZROWS, 2 * slots], F32)
            nc.vector.memset(zsb, 0.0)

            def emit_heads(pw, p_h2, p_pieces, p_slots):
                """Head matmuls + per-slot pooling for window pw (lagged one
                window so the PE-only head phase overlaps next window's L1
                activations instead of starving the scalar engine)."""
                z = [
                    psz.tile([ZROWS, PIECE], F32, tag="z", name=f"z{pw}_{q}")
                    for q in range(len(p_pieces))
                ]
                for q in range(len(p_pieces)):
                    nc.vector.memset(z[q], 0.0)
                for j in range(12):
                    st = 32 * (j % 4)
                    for q, (a, b) in enumerate(p_pieces):
                        nc.tensor.matmul(
                            z[q][st : st + ZD, : b - a],
                            whbdt[:, j * ZD : (j + 1) * ZD],
                            p_h2[j][:, a:b],
                            start=(j < 4),
                            stop=(j >= 8),
                            tile_position=(0, st),
                            skip_group_check=True,
                        )
                # a slot may straddle the two pieces -> two partials, host adds
                for s, off, L in p_slots:
                    q0 = off // PIECE
                    q1 = (off + L - 1) // PIECE
                    if q0 == q1:
                        nc.vector.tensor_reduce(
                            zsb[:, s : s + 1],
                            z[q0][:, off - q0 * PIECE : off - q0 * PIECE + L],
                            axis=mybir.AxisListType.X,
                            op=mybir.AluOpType.add,
                        )
                    else:
                        nc.vector.tensor_reduce(
                            zsb[:, s : s + 1],
                            z[q0][:, off - q0 * PIECE :],
                            axis=mybir.AxisListType.X,
                            op=mybir.AluOpType.add,
                        )
                        nc.vector.tensor_reduce(
                            zsb[:, slots + s : slots + s + 1],
                            z[q1][:, : off + L - q1 * PIECE],
                            axis=mybir.AxisListType.X,
                            op=mybir.AluOpType.add,
                        )

            prev = None
            for w in range(nw):
                used = win_cols[w]
                pieces = [(p, min(p + PIECE, used)) for p in range(0, used, PIECE)]
                xw = xp.tile([73, WIN], BF16, tag="xw")
                for r in range(3):
                    nc.sync.dma_start(
                        xw[32 * r : 32 * r + DIN, :used],
                        xin[:, w * WIN : w * WIN + used],
                    )
                # layer 1 row-packed: strips r=0..2 run concurrently on PE
                h1 = [None] * 12
                for g in range(4):
                    pss = []
                    for r in range(3):
                        j = 3 * g + r
                        ps = psm.tile([128, WIN], F32, tag="ps", name=f"ps1_{w}_{j}")
                        for a, b in pieces:
                            nc.tensor.matmul(
                                ps[:, a:b],
                                w1t[32 * r : 32 * r + DIN, 128 * g : 128 * (g + 1)],
                                xw[32 * r : 32 * r + DIN, a:b],
                                start=True,
                                stop=True,
                                tile_position=(32 * r, 0),
                            )
                        pss.append(ps)
                    for r in range(3):
                        j = 3 * g + r
                        t = h1p.tile([128, WIN], BF16, tag="h1", name=f"h1_{w}_{j}")
                        nc.scalar.activation(
                            t[:, :used], pss[r][:, :used], act, bias=b1t[:, j : j + 1]
                        )
                        h1[j] = t
                # lagged heads+pooling of the previous window overlap here
                if prev is not None:
                    emit_heads(*prev)
                # layer 2
                h2 = [None] * 12
                for n in range(NNETS):
                    for mo in range(2):
                        j = 2 * n + mo
                        ps = psm.tile([128, WIN], F32, tag="ps", name=f"ps2_{w}_{j}")
                        c0 = n * 256 + mo * 128
                        for k in range(2):
                            for a, b in pieces:
                                nc.tensor.matmul(
                                    ps[:, a:b],
                                    w2t[k][:, c0 : c0 + 128],
                                    h1[2 * n + k][:, a:b],
                                    start=(k == 0),
                                    stop=(k == 1),
                                    skip_group_check=True,
                                )
                        t = h2p.tile([128, WIN], BF16, tag="h2", name=f"h2_{w}_{j}")
                        nc.scalar.activation(
                            t[:, :used], ps[:, :used], act, bias=b2t[:, j : j + 1]
                        )
                        h2[j] = t
                prev = (w, h2, pieces, slots_per_win[w])
            emit_heads(*prev)
            nc.sync.dma_start(outt[:, :], zsb)
    nc.compile()
    return nc


# ----------------------------------------------------------------------------
# Host-side weight packing
# ----------------------------------------------------------------------------


def pack_weights(ins):
    W1s = [ins["router_W1"]] + [ins["e_W1"][i] for i in range(2)] + [
        ins["d_W1"][i] for i in range(3)
    ]
    W2s = [ins["router_W2"]] + [ins["e_W2"][i] for i in range(2)] + [
        ins["d_W2"][i] for i in range(3)
    ]
    Whs = [ins["router_Wh"]] + [ins["e_Wh"][i] for i in range(2)] + [
        ins["d_Wh"][i] for i in range(3)
    ]
    b1s = [ins["router_b1"]] + [ins["e_b1"][i] for i in range(2)] + [
        ins["d_b1"][i] for i in range(3)
    ]
    b2s = [ins["router_b2"]] + [ins["e_b2"][i] for i in range(2)] + [
        ins["d_b2"][i] for i in range(3)
    ]
    bhs = [ins["router_bh"]] + [ins["e_bh"][i] for i in range(2)] + [
        ins["d_bh"][i] for i in range(3)
    ]
    f = lambda a: np.ascontiguousarray(np.asarray(a, np.float32))
    W1cat = np.concatenate([f(w) for w in W1s], axis=1)  # [9, 1536]
    W1stack = np.zeros((73, 512), np.float32)
    for j in range(12):
        g, r = j // 3, j % 3
        W1stack[32 * r : 32 * r + DIN, 128 * g : 128 * (g + 1)] = W1cat[
            :, 128 * j : 128 * (j + 1)
        ]
    w2a = np.concatenate([f(w)[0:128, :] for w in W2s], axis=1)  # [128, 1536]
    w2b = np.concatenate([f(w)[128:256, :] for w in W2s], axis=1)
    b1cat = np.concatenate([f(b) for b in b1s])  # [1536]
    b2cat = np.concatenate([f(b) for b in b2s])
    bhcat = np.concatenate([f(b) for b in bhs])  # [19]
    whbd = np.zeros((128, 12 * ZD), np.float32)
    for j in range(12):
        nt, hj = j // 2, j % 2
        whbd[:, j * ZD + ZOFF[nt] : j * ZD + ZOFF[nt] + ZDIMS[nt]] = f(Whs[nt])[
            hj * 128 : (hj + 1) * 128, :
        ]
    b1t = b1cat.reshape(12, 128).T.copy()  # [128, 12]
    b2t = b2cat.reshape(12, 128).T.copy()
    # pad-column contribution (exact; zero when biases are zero)
    h1c = _gelu(b1cat)
    zc = np.zeros(ZD, np.float32)
    for n in range(NNETS):
        a2c = h1c[n * 256 : (n + 1) * 256] @ f(W2s[n]) + f(b2s[n])
        h2c = _gelu(a2c)
        zc[ZOFF[n] : ZOFF[n] + ZDIMS[n]] = h2c @ f(Whs[n])
    bf = lambda a: a.astype(NPBF16)
    return dict(
        w1=bf(W1stack), w2a=bf(w2a), w2b=bf(w2b), whbd=bf(whbd), b1=b1t, b2=b2t,
        bhcat=bhcat,
        zc=zc, W2s=[f(w) for w in W2s], Whs=[f(w) for w in Whs],
        b2s=[f(b) for b in b2s],
    )


def build_xall(x, batch_ids, lay):
    """Scatter points into per-core feature-major padded streams [8, 9, S]."""
    counts = np.bincount(batch_ids, minlength=B)
    seg_start = np.zeros(B, np.int64)
    np.cumsum(counts[:-1], out=seg_start[1:])
    rank = np.empty(B, np.int64)
    rank[lay["ev"].reshape(-1)] = np.arange(B)
    r = rank[batch_ids]
    s = r // N_CORES
    c = r % N_CORES
    pos = np.arange(N_PTS) - seg_start[batch_ids]
    col = lay["slot_win"][s] * WIN + lay["slot_off"][s] + pos
    S = lay["nw"] * WIN
    xall = np.zeros((N_CORES, DIN, S), NPBF16)
    xall[c, :, col] = x.astype(NPBF16)
    return xall


# ----------------------------------------------------------------------------
# Host-side final mixing (exactly mirrors the reference)
# ----------------------------------------------------------------------------


def mix_outputs(y):
    """y: [B, 19] per-event head outputs -> [B, 11] model output."""
    y = y.astype(np.float32)
    morph = y[:, 0:6]
    m = morph - morph.max(axis=1, keepdims=True)
    e = np.exp(m)
    probs = e / e.sum(axis=1, keepdims=True)
    probs = np.maximum(probs, np.float32(1e-6))
    p_cont = probs[:, [0, 1]].sum(1, keepdims=True)
    p_uncont = probs[:, [2, 3, 5]].sum(1, keepdims=True)
    energy = p_cont * y[:, 6:8] + p_uncont * y[:, 8:10]
    p_cas = probs[:, 0:1]
    p_track = probs[:, [1, 2, 3, 5]].sum(1, keepdims=True)
    gate = 1.0 / (1.0 + np.exp(-(energy[:, 0:1] - np.float32(4.0))))
    dirp = p_cas * y[:, 10:13] + p_track * (
        (1.0 - gate) * y[:, 13:16] + gate * y[:, 16:19]
    )
    return np.concatenate([morph, energy, dirp], axis=1).astype(np.float32)


def postprocess(zsums, lay, wp, counts):
    """zsums: [8][19, SLOTS] device outputs -> [B, 11]."""
    y = np.zeros((B, ZD), np.float32)
    ev = lay["ev"]
    slot_len = lay["slot_len"]
    zc = wp["zc"]
    for c in range(N_CORES):
        zf = zsums[c]  # [115, 2*SLOTS]
        zs = np.zeros((ZD, SLOTS), np.float32)
        for st in range(4):
            zs += zf[32 * st : 32 * st + ZD, :SLOTS] + zf[32 * st : 32 * st + ZD, SLOTS:]
        e = ev[:, c]
        cnt = counts[e].astype(np.float32)
        pad = (slot_len - counts[e]).astype(np.float32)
        yy = (zs.T - pad[:, None] * zc[None, :]) / np.maximum(cnt, 1.0)[:, None]
        y[e] = yy + wp["bhcat"][None, :]
    return mix_outputs(y)


# ----------------------------------------------------------------------------
# Entry point
# ----------------------------------------------------------------------------

_CACHE = {}
_LAST_RESULT = None  # set when KERNEL_TRACE=1; holds BassKernelResults


def kernel(**inputs):
    import os

    global _LAST_RESULT
    from concourse.bass_utils import run_bass_kernel_spmd

    ins = {k: np.asarray(v) for k, v in inputs.items()}
    coords = ins["coords"].astype(np.float32)
    features = ins["features"].astype(np.float32)
    batch_ids = ins["batch_ids"].astype(np.int64)
    x = np.concatenate([coords, features], axis=1)  # [N, 9]

    counts = np.bincount(batch_ids, minlength=B)
    lay = build_layout(counts)
    wp = pack_weights(ins)
    xall = build_xall(x, batch_ids, lay)

    key = (lay["nw"], tuple(map(tuple, (tuple(w) for w in lay["slots_per_win"]))))
    key = (key, tuple(lay["win_cols"]))
    if key not in _CACHE:
        _CACHE[key] = build_program(
            lay["nw"], lay["slots_per_win"], win_cols=lay["win_cols"]
        )
    nc = _CACHE[key]

    shared = {
        k: wp[k] for k in ("w1", "w2a", "w2b", "whbd", "b1", "b2")
    }
    in_maps = [dict(shared, xin=np.ascontiguousarray(xall[c])) for c in range(N_CORES)]
    trace = bool(int(os.environ.get("KERNEL_TRACE", "0")))
    res = run_bass_kernel_spmd(
        nc, in_maps, core_ids=list(range(N_CORES)), trace=trace
    )
    _LAST_RESULT = res
    zsums = [res.results[c]["zsum"] for c in range(N_CORES)]
    return postprocess(zsums, lay, wp, counts)


# revision 17
# speedup vs baseline: 1.4506x; 1.1078x over previous
"""Trainium2 Bass kernel for nn_NeptuneMoEModel (moe_routing).

Model: 6 small MLPs (router + 2 energy experts + 3 direction experts) over
N=262144 points -> segment-mean-pool into B=1024 events -> tiny per-event
head/mixing math.

Strategy (8 NeuronCores, SPMD, data-parallel over events):
  - Events sorted by point count and round-robin assigned to cores so slot s
    on every core holds a similarly-sized event; slot lengths are uniform
    across cores (required: one program for all 8 cores).
  - Slots first-fit packed into 1024-column "windows" (= 2 PSUM banks).
  - Feature-major layout on device: x as [9, S]; layer1 = fused [9, 1536]
    matmul, layer2 = 6x [256,256], all in float32r (1 cyc/row at N=512).
  - All 6 heads fused into one block-diagonal [1536 -> 19] matmul that
    accumulates in PSUM (12 accumulating matmuls per window); pooling then
    reduces only [19, L] per event on the vector engine.
  - gelu (tanh approx, matches jax.nn.gelu) via big [128, 1024] scalar-engine
    activations reading PSUM directly, per-partition bias APs.
  - Host: pad-correction (exact, general for nonzero biases), divide by
    counts, head biases, softmax/gating mixing - all O(B*19) numpy.
"""

import sys

sys.path.insert(0, "/opt/trn_rl_repo")

import numpy as np

import concourse.bass as bass
import concourse.mybir as mybir
import concourse.tile as tile
from concourse import bacc

N_CORES = 8
B = 1024
N_PTS = 262144
DIN = 9
H = 256
NNETS = 6
ZDIMS = [6, 2, 2, 3, 3, 3]
ZOFF = [0, 6, 8, 10, 13, 16]
ZD = 19
WIN = 2048
PIECE = 512
SLOTS = B // N_CORES  # 128
F32 = mybir.dt.float32
BF16 = mybir.dt.bfloat16
try:
    import ml_dtypes

    NPBF16 = ml_dtypes.bfloat16
except ImportError:  # pragma: no cover
    NPBF16 = None
GELU = mybir.ActivationFunctionType.Gelu_apprx_tanh


def _gelu(x):
    """jax.nn.gelu(approximate=True) in numpy/fp32."""
    x = np.asarray(x, np.float32)
    c = np.float32(np.sqrt(2.0 / np.pi))
    return (0.5 * x * (1.0 + np.tanh(c * (x + 0.044715 * x * x * x)))).astype(
        np.float32
    )


# ----------------------------------------------------------------------------
# Layout: event -> (core, slot); slots -> windows
# ----------------------------------------------------------------------------


def build_layout(counts):
    counts = np.asarray(counts)
    order = np.argsort(-counts, kind="stable")
    ev = order.reshape(SLOTS, N_CORES)  # ev[s, c] = event id
    slot_len = counts[ev].max(1)
    slot_len = np.maximum(((slot_len + 3) // 4) * 4, 4).astype(np.int64)
    assert slot_len.max() <= WIN
    # first-fit (slot_len is non-increasing -> this is first-fit-decreasing)
    win_used = []
    slot_win = np.zeros(SLOTS, np.int64)
    slot_off = np.zeros(SLOTS, np.int64)
    for s in range(SLOTS):
        L = int(slot_len[s])
        for w in range(len(win_used)):
            if win_used[w] + L <= WIN:
                slot_win[s] = w
                slot_off[s] = win_used[w]
                win_used[w] += L
                break
        else:
            slot_win[s] = len(win_used)
            slot_off[s] = 0
            win_used.append(L)
    nw = len(win_used)
    slots_per_win = [[] for _ in range(nw)]
    for s in range(SLOTS):
        slots_per_win[slot_win[s]].append(
            (s, int(slot_off[s]), int(slot_len[s]))
        )
    win_cols = [min(WIN, ((u + 7) // 8) * 8) for u in win_used]
    return dict(
        ev=ev,
        slot_len=slot_len,
        slot_win=slot_win,
        slot_off=slot_off,
        nw=nw,
        slots_per_win=slots_per_win,
        win_cols=win_cols,
    )


# ----------------------------------------------------------------------------
# Device program
# ----------------------------------------------------------------------------


def build_program(nw, slots_per_win, win_cols=None, slots=SLOTS, act=GELU):
    """v3: no on-device heads. Layer1+layer2 matmuls (bf16, fp32 psum),
    gelu on big [128, <=2048] scalar-engine activations, and per-(slot,
    feature-tile) pooling via vector-engine reduces straight from SBUF h2.
    The [1536 -> 19] head runs on host on pooled vectors."""
    nc = bacc.Bacc(None, target_bir_lowering=False)
    if win_cols is None:
        win_cols = [WIN] * nw
    S = nw * WIN
    xin = nc.dram_tensor("xin", [DIN, S], BF16, kind="ExternalInput")
    w1 = nc.dram_tensor("w1", [DIN, 12 * 128], BF16, kind="ExternalInput")
    w2a = nc.dram_tensor("w2a", [128, NNETS * 256], BF16, kind="ExternalInput")
    w2b = nc.dram_tensor("w2b", [128, NNETS * 256], BF16, kind="ExternalInput")
    b1 = nc.dram_tensor("b1", [128, 12], F32, kind="ExternalInput")
    b2 = nc.dram_tensor("b2", [128, 12], F32, kind="ExternalInput")
    outt = nc.dram_tensor("zsum", [128, 12 * slots], F32, kind="ExternalOutput")

    with tile.TileContext(nc) as tc:
        with (
            tc.tile_pool(name="wts", bufs=1) as wts,
            tc.tile_pool(name="xp", bufs=3) as xp,
            tc.tile_pool(name="h1p", bufs=14) as h1p,
            tc.tile_pool(name="h2p", bufs=4) as h2p,
            tc.tile_pool(name="op", bufs=1) as op,
            tc.tile_pool(name="psm", bufs=2, space="PSUM") as psm,
        ):
            w1t = wts.tile([DIN, 12 * 128], BF16)
            nc.sync.dma_start(w1t, w1[:, :])
            w2t = [wts.tile([128, NNETS * 256], BF16, name=f"w2_{k}") for k in range(2)]
            nc.sync.dma_start(w2t[0], w2a[:, :])
            nc.sync.dma_start(w2t[1], w2b[:, :])
            b1t = wts.tile([128, 12], F32)
            nc.sync.dma_start(b1t, b1[:, :])
            b2t = wts.tile([128, 12], F32)
            nc.sync.dma_start(b2t, b2[:, :])
            zsb = op.tile([128, 12 * slots], F32)

            for w in range(nw):
                used = win_cols[w]
                pieces = [(p, min(p + PIECE, used)) for p in range(0, used, PIECE)]
                xw = xp.tile([DIN, WIN], BF16, tag="xw")
                nc.sync.dma_start(
                    xw[:, :used], xin[:, w * WIN : w * WIN + used]
                )
                h1 = [None] * 12
                for j in range(12):
                    ps = psm.tile([128, WIN], F32, tag="ps", name=f"ps1_{w}_{j}")
                    for a, b in pieces:
                        nc.tensor.matmul(
                            ps[:, a:b],
                            w1t[:, j * 128 : (j + 1) * 128],
                            xw[:, a:b],
                            start=True,
                            stop=True,
                        )
                    t = h1p.tile([128, WIN], BF16, tag="h1", name=f"h1_{w}_{j}")
                    nc.scalar.activation(
                        t[:, :used], ps[:, :used], act, bias=b1t[:, j : j + 1]
                    )
                    h1[j] = t
                for n in range(NNETS):
                    for mo in range(2):
                        j = 2 * n + mo
                        ps = psm.tile([128, WIN], F32, tag="ps", name=f"ps2_{w}_{j}")
                        c0 = n * 256 + mo * 128
                        for k in range(2):
                            for a, b in pieces:
                                nc.tensor.matmul(
                                    ps[:, a:b],
                                    w2t[k][:, c0 : c0 + 128],
                                    h1[2 * n + k][:, a:b],
                                    start=(k == 0),
                                    stop=(k == 1),
                                    skip_group_check=True,
                                )
                        t = h2p.tile([128, WIN], BF16, tag="h2", name=f"h2_{w}_{j}")
                        nc.scalar.activation(
                            t[:, :used], ps[:, :used], act, bias=b2t[:, j : j + 1]
                        )
                        # pool: per-slot sums of this feature tile (DVE, hidden)
                        for s, off, L in slots_per_win[w]:
                            nc.vector.tensor_reduce(
                                zsb[:, j * slots + s : j * slots + s + 1],
                                t[:, off : off + L],
                                axis=mybir.AxisListType.X,
                                op=mybir.AluOpType.add,
                            )
            nc.sync.dma_start(outt[:, :], zsb)
    nc.compile()
    return nc


# ----------------------------------------------------------------------------
# Host-side weight packing
# ----------------------------------------------------------------------------


def pack_weights(ins):
    W1s = [ins["router_W1"]] + [ins["e_W1"][i] for i in range(2)] + [
        ins["d_W1"][i] for i in range(3)
    ]
    W2s = [ins["router_W2"]] + [ins["e_W2"][i] for i in range(2)] + [
        ins["d_W2"][i] for i in range(3)
    ]
    Whs = [ins["router_Wh"]] + [ins["e_Wh"][i] for i in range(2)] + [
        ins["d_Wh"][i] for i in range(3)
    ]
    b1s = [ins["router_b1"]] + [ins["e_b1"][i] for i in range(2)] + [
        ins["d_b1"][i] for i in range(3)
    ]
    b2s = [ins["router_b2"]] + [ins["e_b2"][i] for i in range(2)] + [
        ins["d_b2"][i] for i in range(3)
    ]
    bhs = [ins["router_bh"]] + [ins["e_bh"][i] for i in range(2)] + [
        ins["d_bh"][i] for i in range(3)
    ]
    f = lambda a: np.ascontiguousarray(np.asarray(a, np.float32))
    W1cat = np.concatenate([f(w) for w in W1s], axis=1)  # [9, 1536]
    w2a = np.concatenate([f(w)[0:128, :] for w in W2s], axis=1)  # [128, 1536]
    w2b = np.concatenate([f(w)[128:256, :] for w in W2s], axis=1)
    b1cat = np.concatenate([f(b) for b in b1s])  # [1536]
    b2cat = np.concatenate([f(b) for b in b2s])
    bhcat = np.concatenate([f(b) for b in bhs])  # [19]
    b1t = b1cat.reshape(12, 128).T.copy()  # [128, 12]
    b2t = b2cat.reshape(12, 128).T.copy()
    # pad-column contribution per h2 feature (exact; zero when biases zero)
    h1c = _gelu(b1cat)
    h2c_cat = np.zeros(1536, np.float32)
    for n in range(NNETS):
        a2c = h1c[n * 256 : (n + 1) * 256] @ f(W2s[n]) + f(b2s[n])
        h2c_cat[n * 256 : (n + 1) * 256] = _gelu(a2c)
    bf = lambda a: a.astype(NPBF16)
    return dict(
        w1=bf(W1cat), w2a=bf(w2a), w2b=bf(w2b), b1=b1t, b2=b2t,
        bhcat=bhcat, h2c_cat=h2c_cat, Whs=[f(w) for w in Whs],
    )


def build_xall(x, batch_ids, lay):
    """Scatter points into per-core feature-major padded streams [8, 9, S]."""
    counts = np.bincount(batch_ids, minlength=B)
    seg_start = np.zeros(B, np.int64)
    np.cumsum(counts[:-1], out=seg_start[1:])
    rank = np.empty(B, np.int64)
    rank[lay["ev"].reshape(-1)] = np.arange(B)
    r = rank[batch_ids]
    s = r // N_CORES
    c = r % N_CORES
    pos = np.arange(N_PTS) - seg_start[batch_ids]
    col = lay["slot_win"][s] * WIN + lay["slot_off"][s] + pos
    S = lay["nw"] * WIN
    xall = np.zeros((N_CORES, DIN, S), NPBF16)
    xall[c, :, col] = x.astype(NPBF16)
    return xall


# ----------------------------------------------------------------------------
# Host-side final mixing (exactly mirrors the reference)
# ----------------------------------------------------------------------------


def mix_outputs(y):
    """y: [B, 19] per-event head outputs -> [B, 11] model output."""
    y = y.astype(np.float32)
    morph = y[:, 0:6]
    m = morph - morph.max(axis=1, keepdims=True)
    e = np.exp(m)
    probs = e / e.sum(axis=1, keepdims=True)
    probs = np.maximum(probs, np.float32(1e-6))
    p_cont = probs[:, [0, 1]].sum(1, keepdims=True)
    p_uncont = probs[:, [2, 3, 5]].sum(1, keepdims=True)
    energy = p_cont * y[:, 6:8] + p_uncont * y[:, 8:10]
    p_cas = probs[:, 0:1]
    p_track = probs[:, [1, 2, 3, 5]].sum(1, keepdims=True)
    gate = 1.0 / (1.0 + np.exp(-(energy[:, 0:1] - np.float32(4.0))))
    dirp = p_cas * y[:, 10:13] + p_track * (
        (1.0 - gate) * y[:, 13:16] + gate * y[:, 16:19]
    )
    return np.concatenate([morph, energy, dirp], axis=1).astype(np.float32)


def postprocess(zsums, lay, wp, counts):
    """zsums: [8][128, 12*SLOTS] pooled-h2 sums -> [B, 11]."""
    y = np.zeros((B, ZD), np.float32)
    ev = lay["ev"]
    slot_len = lay["slot_len"]
    h2c = wp["h2c_cat"]
    for c in range(N_CORES):
        zf = zsums[c]  # [128, 12*SLOTS]; col j*SLOTS+s = features of tile j
        pooled = (
            zf.reshape(128, 12, SLOTS).transpose(2, 1, 0).reshape(SLOTS, 1536)
        )
        e = ev[:, c]
        cnt = counts[e].astype(np.float32)
        pad = (slot_len - counts[e]).astype(np.float32)
        pooled = (pooled - pad[:, None] * h2c[None, :]) / np.maximum(cnt, 1.0)[
            :, None
        ]
        yy = np.zeros((SLOTS, ZD), np.float32)
        for n in range(NNETS):
            yy[:, ZOFF[n] : ZOFF[n] + ZDIMS[n]] = (
                pooled[:, n * 256 : (n + 1) * 256] @ wp["Whs"][n]
            )
        y[e] = yy + wp["bhcat"][None, :]
    return mix_outputs(y)


# ----------------------------------------------------------------------------
# Entry point
# ----------------------------------------------------------------------------

_CACHE = {}
_LAST_RESULT = None  # set when KERNEL_TRACE=1; holds BassKernelResults


def kernel(**inputs):
    import os

    global _LAST_RESULT
    from concourse.bass_utils import run_bass_kernel_spmd

    ins = {k: np.asarray(v) for k, v in inputs.items()}
    coords = ins["coords"].astype(np.float32)
    features = ins["features"].astype(np.float32)
    batch_ids = ins["batch_ids"].astype(np.int64)
    x = np.concatenate([coords, features], axis=1)  # [N, 9]

    counts = np.bincount(batch_ids, minlength=B)
    lay = build_layout(counts)
    wp = pack_weights(ins)
    xall = build_xall(x, batch_ids, lay)

    key = (lay["nw"], tuple(map(tuple, (tuple(w) for w in lay["slots_per_win"]))))
    key = (key, tuple(lay["win_cols"]))
    if key not in _CACHE:
        _CACHE[key] = build_program(
            lay["nw"], lay["slots_per_win"], win_cols=lay["win_cols"]
        )
    nc = _CACHE[key]

    shared = {
        k: wp[k] for k in ("w1", "w2a", "w2b", "b1", "b2")
    }
    in_maps = [dict(shared, xin=np.ascontiguousarray(xall[c])) for c in range(N_CORES)]
    trace = bool(int(os.environ.get("KERNEL_TRACE", "0")))
    res = run_bass_kernel_spmd(
        nc, in_maps, core_ids=list(range(N_CORES)), trace=trace
    )
    _LAST_RESULT = res
    zsums = [res.results[c]["zsum"] for c in range(N_CORES)]
    return postprocess(zsums, lay, wp, counts)
